# revision 27
# baseline (speedup 1.0000x reference)
"""Multi-head attention (N=2, L=2048, D=1024, H=16) on 8 NeuronCores.

Sharding: core c -> (batch n = c // 4, head group g = c % 4, 4 heads each).
Each core computes Q/K/V projections for its 4 heads, attention, and its
slice of the output projection. Host sums the 4 partial output projections
per batch and adds bo. HW exec ~210-213us (baseline was ~260-270us).

Design (v7), from iterative HW profiling:
- Host-blocked qt-major x layouts ([QT*128, DC*512], 8KB DMA lines) so
  inputs arrive in consumption order (the prior full-row layout starved
  PV of V until ~58us). The prior session's NaN-on-profiled-core fear
  about this layout did NOT reproduce.
- Minimal prelude (qproj(0,0)+kproj(0,0) only) then single-head rounds:
  per (qb, hp, sk, hh): S^T = 2 MMs into one [128,1024] fp32 psum tile
  (ring 2), one exp [128,1024] (ScalarE, scale=1/32 folded), PV = 2 MMs
  (M=65: V dims + ones column accumulating the softmax denominator).
  Ring granularity == round granularity keeps the exp stream fed.
- PV lags TWO rounds behind S^T so its exp finished >1 round earlier —
  the PE never waits on a fresh semaphore (this removed ~160ns/MM of
  exposed drain: PV means fell 253->222ns).
- Head-pair outer loop (hp): only 2 PV accumulators live -> PSUM fits:
  s(2x2 banks) + acc(2x1) + o(2x1) = 8 banks, leaving the "o" ring for
  interleaved filler matmuls. All projection/out-proj work is emitted as
  deadline-scheduled round fillers (PE FIFO = emission order, so every
  emitted op must have its deps already emitted).
- Out-proj packs head pairs: oN2[hp] [128,512] holds both heads' dims ->
  K=128 fully used, 8 MMs/qb instead of 16. The odd head's normalized
  rows are written by DVE tensor_tensor at partition offset 64 (works on
  this stack; single-partition DVE ops at an offset do NOT).
- Reciprocal on DVE: bc matmul broadcasts the RAW denominator row (K=1
  matmul from partition 64), reciprocal_approx_fast on [64,512] (the
  exact InstReciprocal is 5x slower: 3.2us/tile!), one tensor_tensor
  multiply. ScalarE runs exps only; no Ln/Exp act-table tricks needed.
- HAM keepalive: warmup matmuls on memset tiles during the DMA lead-in,
  plus dummy matmuls bridging the endgame dependency stalls (without
  them the final out-proj runs at the cold 427ns clock, +3.4us).
- Endgame out-proj alternates psum tags "o"/"s" (4 tiles in flight) and
  CASTs alternate DVE/ScalarE so the copy-out is not DVE-bound.
Remaining structure: PE busy ~186us (floor ~174-177: S^T and PV are
output/stream-bound at N=512; fp8 rejected — P-noise transfers ~1:1 to
output rel err, ~1.3-3% vs the 2e-2 gate), ScalarE exp stream 135us,
~8us runtime head + ~7us teardown tail are fixed. Exp batching to
[128,2048] would save ~14us ScalarE but needs 2 more PSUM banks than
exist. Run-to-run variance is +-1.5% (HAM phase / power states).
"""
import os
import sys
import types

import numpy as np

N_BATCH = 2
L = 2048
D = 1024
H = 16
HD = 64
CORES = 8
GH = 4            # heads per core
DG = GH * HD      # 256 = projected dims per core
QB = 512          # q block
KT = L // 128     # 16 k tiles
QT = L // QB      # 4 q blocks
DC = D // 128     # 8 din chunks
SCALE = 1.0 / 32.0  # 1/sqrt(D)
SHIFT_MODE = "dve"  # "dve": direct offset write; "pe": identity matmul shift


def _install_ntff_hook():
    """The image's antenv stub lacks axon_hooks; shim it so trace=True works."""
    if "antenv.axon_hooks" in sys.modules:
        return
    mod = types.ModuleType("antenv.axon_hooks")
    mod._hook = None
    mod.set_axon_ntff_profile_hook = lambda h: setattr(mod, "_hook", h)
    mod.get_axon_ntff_profile_hook = lambda: mod._hook
    sys.modules["antenv.axon_hooks"] = mod
    try:
        from trn_agent_boot.trn_boot import _ntff_profile_via_ctypes
        mod._hook = _ntff_profile_via_ctypes("/opt/axon/libaxon_pjrt.so")
    except Exception:
        mod._hook = None


_install_ntff_hook()

import concourse.bacc as bacc
import concourse.mybir as mybir
import concourse.tile as tile
from concourse.bass_utils import run_bass_kernel_spmd

F32 = mybir.dt.float32
F16 = mybir.dt.float16
AF = mybir.ActivationFunctionType
MULT = mybir.AluOpType.mult

_CACHE = {}


def _build(use_bias, use_mask):
    key = (use_bias, use_mask)
    if key in _CACHE:
        return _CACHE[key]

    nc = bacc.Bacc("TRN2", debug=False, num_devices=CORES)

    xqb = nc.dram_tensor("xqb", [QT * 128, DC * 512], F16, kind="ExternalInput").ap()
    xkb = nc.dram_tensor("xkb", [QT * 128, DC * 512], F16, kind="ExternalInput").ap()
    xvb = nc.dram_tensor("xvb", [QT * 128, DC * 512], F16, kind="ExternalInput").ap()
    aq = nc.dram_tensor("aq", [128, DC * DG], F16, kind="ExternalInput").ap()
    ak = nc.dram_tensor("ak", [128, DC * DG], F16, kind="ExternalInput").ap()
    av = nc.dram_tensor("av", [128, DC * DG], F16, kind="ExternalInput").ap()
    bo = nc.dram_tensor("bo", [128, 2 * D], F16, kind="ExternalInput").ap()
    bq = nc.dram_tensor("bq", [1, DG], F16, kind="ExternalInput").ap()
    bk = nc.dram_tensor("bk", [1, DG], F16, kind="ExternalInput").ap()
    bv = nc.dram_tensor("bv", [1, DG], F16, kind="ExternalInput").ap()
    eye = nc.dram_tensor("eye", [64, 64], F16, kind="ExternalInput").ap()
    maskf = nc.dram_tensor("maskf", [128, KT], F32, kind="ExternalInput").ap()
    outp = nc.dram_tensor("outp", [L, D], F16, kind="ExternalOutput").ap()

    with tile.TileContext(nc) as tc:
        _emit(nc, tc, dict(xqb=xqb, xkb=xkb, xvb=xvb, aq=aq, ak=ak, av=av,
                           bo=bo, bq=bq, bk=bk, bv=bv, eye=eye, maskf=maskf,
                           outp=outp),
              use_bias, use_mask)
    nc.compile()
    _CACHE[key] = nc
    return nc


def _emit(nc, tc, t, use_bias, use_mask):
    from contextlib import ExitStack
    ctx = ExitStack()
    with ctx:
        sb_w = ctx.enter_context(tc.tile_pool(name="sb_w", bufs=1))
        sb_qkv = ctx.enter_context(tc.tile_pool(name="sb_qkv", bufs=1))
        sb_pt = ctx.enter_context(tc.tile_pool(name="sb_pt", bufs=4))
        sb_n = ctx.enter_context(tc.tile_pool(name="sb_n", bufs=4))
        sb_out = ctx.enter_context(tc.tile_pool(name="sb_out", bufs=3))
        ps = ctx.enter_context(tc.tile_pool(name="ps", bufs=2, space="PSUM"))

        # ---- resident tiles ----
        aq_t = sb_w.tile([128, DC, DG], F16, tag="aq")
        ak_t = sb_w.tile([128, DC, DG], F16, tag="ak")
        av_t = sb_w.tile([128, DC, DG], F16, tag="av")
        bo_t = sb_w.tile([128, 2, D], F16, tag="bo")
        ones_t = sb_w.tile([128, 512], F16, tag="ones")
        eye_t = sb_w.tile([64, 64], F16, tag="eye")
        xq_res = sb_w.tile([128, QT, DC * 512], F16, tag="xq")
        xk_res = sb_w.tile([128, QT, DC * 512], F16, tag="xk")
        xv_res = sb_w.tile([128, QT, DC * 512], F16, tag="xv")
        KT_sb = [sb_qkv.tile([128, L], F16, tag=f"kt{m}", name=f"KTm{m}")
                 for m in range(2)]
        QT_z = [sb_qkv.tile([128, L], F16, tag=f"qz{h}", name=f"QTz{h}")
                for h in range(GH)]
        V1 = sb_qkv.tile([128, KT, GH, HD + 1], F16, tag="v1")
        # oN2[qb%2][hp]: packed normalized heads for the out-projection
        oN2 = [[sb_qkv.tile([128, 512], F16, tag=f"oN{b}{hp}",
                            name=f"oN{b}{hp}") for hp in range(2)]
               for b in range(2)]

        # ---- warmup tiles (no DMA deps; HAM ramp during input stream) ----
        warm_w = sb_w.tile([128, 128], F16, tag="warmw")
        warm_x = sb_w.tile([128, 512], F16, tag="warmx")
        nc.vector.memset(warm_w, 0.0)
        nc.vector.memset(warm_x, 0.0)
        nc.vector.memset(ones_t, 1.0)
        # ACT table warmup (exp)
        warm_a = sb_w.tile([1, 32], F32, tag="warma")
        nc.vector.memset(warm_a, 1.0)
        warm_b = sb_w.tile([1, 32], F32, tag="warmb")
        nc.scalar.activation(out=warm_b, in_=warm_a, func=AF.Exp)

        for h in range(GH):
            z0 = 0 if h % 2 else 64
            nc.vector.memset(QT_z[h][z0:z0 + 64, :], 0.0)

        # ---- input DMAs: one priority-ordered queue (sync) ----
        # First tiles split in halves so qproj/kproj start ~2us earlier.
        def dma_x(res, src, qt, half=None):
            if half is None:
                nc.sync.dma_start(out=res[:, qt, :],
                                  in_=src[qt * 128:(qt + 1) * 128, :])
            else:
                h0 = half * (DC // 2) * 512
                h1 = h0 + (DC // 2) * 512
                nc.sync.dma_start(out=res[:, qt, h0:h1],
                                  in_=src[qt * 128:(qt + 1) * 128, h0:h1])

        dma_x(xq_res, t["xqb"], 0, 0)
        nc.sync.dma_start(out=aq_t, in_=t["aq"].rearrange("p (c d) -> p c d", c=DC))
        dma_x(xq_res, t["xqb"], 0, 1)
        nc.sync.dma_start(out=ak_t, in_=t["ak"].rearrange("p (c d) -> p c d", c=DC))
        dma_x(xk_res, t["xkb"], 0, 0)
        dma_x(xk_res, t["xkb"], 0, 1)
        nc.sync.dma_start(out=av_t, in_=t["av"].rearrange("p (c d) -> p c d", c=DC))
        dma_x(xv_res, t["xvb"], 0, 0)
        dma_x(xv_res, t["xvb"], 0, 1)
        for qt in range(1, QT):
            dma_x(xk_res, t["xkb"], qt)
            dma_x(xv_res, t["xvb"], qt)
            dma_x(xq_res, t["xqb"], qt)
        nc.sync.dma_start(out=bo_t, in_=t["bo"].rearrange("p (a d) -> p a d", a=2))
        if SHIFT_MODE == "pe":
            nc.sync.dma_start(out=eye_t, in_=t["eye"])
        if use_mask:
            mask_t = sb_w.tile([128, KT], F32, tag="mask")
            nc.sync.dma_start(out=mask_t, in_=t["maskf"])
        bq_t = bk_t = bv_t = None
        if use_bias:
            bq_t = sb_w.tile([1, DG], F16, tag="bq")
            bk_t = sb_w.tile([1, DG], F16, tag="bk")
            bv_t = sb_w.tile([1, DG], F16, tag="bv")
            nc.sync.dma_start(out=bq_t, in_=t["bq"])
            nc.sync.dma_start(out=bk_t, in_=t["bk"])
            nc.sync.dma_start(out=bv_t, in_=t["bv"])

        # V1 ones column (column HD of every (kt, h) slot)
        if use_mask:
            ones4 = sb_w.tile([128, GH], F32, tag="ones4")
            nc.vector.memset(ones4, 1.0)
            for kt in range(KT):
                nc.vector.tensor_scalar_mul(
                    V1[:, kt, :, HD:HD + 1],
                    ones4.rearrange("p h -> p h 1"), mask_t[:, kt:kt + 1])
        else:
            nc.vector.memset(V1[:, :, :, HD:HD + 1], 1.0)

        # ---- PE warmup: dummy matmuls to ramp HAM while inputs stream ----
        for w in range(10):
            psw = ps.tile([128, 512], F32, tag="o", bufs=2, name=f"psw_{w}")
            nc.tensor.matmul(psw[:, 0:512], warm_w, warm_x,
                             start=True, stop=True)

        # ---- emit helpers ----
        def emit_qproj(qb, p):
            # packed head pair p: one M=128 matmul per c chunk
            psq = ps.tile([128, 512], F32, tag="o", bufs=2, name=f"psq_{qb}_{p}")
            for c in range(DC):
                xsl = xq_res[:, qb, c * 512:(c + 1) * 512]
                nc.tensor.matmul(
                    psq[:, 0:512], aq_t[:, c, p * 128:(p + 1) * 128], xsl,
                    start=(c == 0), stop=(c == DC - 1 and not use_bias))
            if use_bias:
                nc.tensor.matmul(
                    psq[:, 0:512], bq_t[:, p * 128:(p + 1) * 128],
                    ones_t[0:1, :], start=False, stop=True)
            for hh in range(2):
                h = p * 2 + hh
                r0 = 64 * hh
                nc.vector.tensor_copy(
                    QT_z[h][r0:r0 + 64, qb * 512:(qb + 1) * 512],
                    psq[r0:r0 + 64, 0:512])

        def emit_kproj(qt, m):
            psm = ps.tile([128, 512], F32, tag="o", bufs=2, name=f"psk_{qt}_{m}")
            for c in range(DC):
                xsl = xk_res[:, qt, c * 512:(c + 1) * 512]
                nc.tensor.matmul(
                    psm[:, 0:512], ak_t[:, c, m * 128:(m + 1) * 128], xsl,
                    start=(c == 0), stop=(c == DC - 1 and not use_bias))
            if use_bias:
                nc.tensor.matmul(
                    psm[:, 0:512], bk_t[:, m * 128:(m + 1) * 128],
                    ones_t[0:1, :], start=False, stop=True)
            nc.vector.tensor_copy(
                KT_sb[m][:, qt * 512:(qt + 1) * 512], psm[:, 0:512])

        def emit_vproj(ktg, j):
            psv = ps.tile([128, 512], F32, tag="o", bufs=2, name=f"psv_{ktg}_{j}")
            for c in range(DC):
                xsl = xv_res[:, ktg, c * 512:(c + 1) * 512]
                nc.tensor.matmul(
                    psv[:, 0:DG], xsl[:, j * 128:(j + 1) * 128],
                    av_t[:, c, :],
                    start=(c == 0), stop=(c == DC - 1 and not use_bias))
            if use_bias:
                nc.tensor.matmul(
                    psv[:, 0:DG], ones_t[0:1, 0:128], bv_t,
                    start=False, stop=True)
            kt = ktg * 4 + j
            srcv = psv[:, 0:DG].rearrange("p (h d) -> p h d", h=GH)
            if use_mask:
                nc.vector.tensor_scalar_mul(
                    V1[:, kt, :, 0:HD], srcv, mask_t[:, kt:kt + 1])
            else:
                nc.vector.tensor_copy(V1[:, kt, :, 0:HD], srcv)

        # ---- the attention round engine ----
        # round r = (qb, hp, sk, hh): S^T (2 MMs -> pss), exp, then PV of
        # the PREVIOUS round (software pipeline, 1-round lag).
        filler = []           # list of closures, each ~2 matmuls
        fill_debt = [0.0]     # fractional chunks owed

        def pop_filler(n=1.0):
            fill_debt[0] += n
            while fill_debt[0] >= 1.0 and filler:
                filler.pop(0)()
                fill_debt[0] -= 1.0

        def emit_st(qb, hp, sk, hh):
            h = hp * 2 + hh
            pss = ps.tile([128, 1024], F32, tag="s", bufs=2,
                          name=f"pss_{qb}_{sk}_{h}")
            for dk in range(2):
                kt = sk * 2 + dk
                nc.tensor.matmul(
                    pss[:, dk * 512:(dk + 1) * 512],
                    KT_sb[hp][:, kt * 128:(kt + 1) * 128],
                    QT_z[h][:, qb * 512:qb * 512 + QB],
                    start=True, stop=True)
            pt = sb_pt.tile([128, 1024], F16, tag="pt", bufs=6,
                            name=f"pt_{qb}_{sk}_{h}")
            nc.scalar.activation(out=pt, in_=pss, func=AF.Exp, scale=SCALE)
            return pt

        def emit_pv(qb, hp, sk, hh, pt, pso):
            h = hp * 2 + hh
            for dk in range(2):
                kt = sk * 2 + dk
                nc.tensor.matmul(
                    pso[hh][0:HD + 1, :], V1[:, kt, h, :],
                    pt[:, dk * 512:(dk + 1) * 512],
                    start=(kt == 0), stop=(kt == KT - 1))

        def emit_hp_tail(qb, hp, pso):
            # normalize both heads of the pair into oN2[qb%2][hp];
            # the two heads' chains are interleaved so engines pipeline
            on = oN2[qb % 2][hp]
            oTs, bcs, rcps = [], [], []
            for hh in range(2):
                oT = sb_n.tile([HD + 1, 512], F16, tag="oT", bufs=4,
                               name=f"oT_{qb}_{hp}_{hh}")
                nc.vector.tensor_copy(oT, pso[hh][0:HD + 1, :])
                oTs.append(oT)
            for hh in range(2):
                bc = ps.tile([128, 512], F32, tag="o", bufs=2,
                             name=f"bc_{qb}_{hp}_{hh}")
                nc.tensor.matmul(
                    bc[0:64, :], ones_t[64:65, 0:64], oTs[hh][64:65, :],
                    start=True, stop=True, tile_position=(64, 0))
                bcs.append(bc)
            for hh in range(2):
                rcp = sb_n.tile([64, 512], F32, tag="rcp", bufs=2,
                                name=f"rcp_{qb}_{hp}_{hh}")
                nc.vector.reciprocal_approx_fast(out=rcp, in_=bcs[hh][0:64, :])
                rcps.append(rcp)
            for hh in range(2):
                rows = slice(0, 64) if hh == 0 else slice(64, 128)
                nc.vector.tensor_tensor(on[rows, :], oTs[hh][0:64, :],
                                        rcps[hh], op=MULT)

        def emit_outproj_chunk(qb, mq, nb, psout_box, tag="o"):
            # two packed MMs (hp 0,1) accumulating psout, then CAST out
            on_pair = oN2[qb % 2]
            psout = ps.tile([128, 512], F32, tag=tag, bufs=2,
                            name=f"psout_{qb}_{mq}_{nb}")
            for hp in range(2):
                nc.tensor.matmul(
                    psout[:, 0:512],
                    on_pair[hp][:, mq * 128:(mq + 1) * 128],
                    bo_t[:, hp, nb * 512:(nb + 1) * 512],
                    start=(hp == 0), stop=(hp == 1))
            psout_box[nb] = psout

        def emit_outproj(qb, deep=False):
            # returns filler closures: 8 chunks of 2 MMs + CAST/DMA.
            # deep=True (endgame): alternate psum tags "o"/"s" so 4 psout
            # tiles pipeline (the attention rings are drained by then).
            chunks = []
            for mq in range(4):
                ot = sb_out.tile([128, D], F16, tag="ot", name=f"ot_{qb}_{mq}")
                box = {}
                tg0 = "s" if deep and mq % 2 else "o"
                tg1 = "s" if deep and not mq % 2 else "o"

                def mk(qb=qb, mq=mq, ot=ot, box=box, tg0=tg0, tg1=tg1):
                    # endgame CASTs alternate ScalarE/DVE (ScalarE's exp
                    # stream is drained by then) so the copy-out pipelines
                    def cp(dst, src):
                        if deep and mq % 2:
                            nc.scalar.copy(out=dst, in_=src)
                        else:
                            nc.vector.tensor_copy(dst, src)

                    def c0():
                        emit_outproj_chunk(qb, mq, 0, box, tg0)
                        cp(ot[:, 0:512], box[0][:, 0:512])
                        if deep:
                            # endgame: ship each half as soon as it's cast
                            q0 = qb * QB + mq * 128
                            nc.sync.dma_start(
                                out=t["outp"][q0:q0 + 128, 0:512],
                                in_=ot[:, 0:512])

                    def c1():
                        emit_outproj_chunk(qb, mq, 1, box, tg1)
                        cp(ot[:, 512:1024], box[1][:, 0:512])
                        q0 = qb * QB + mq * 128
                        if deep:
                            nc.sync.dma_start(
                                out=t["outp"][q0:q0 + 128, 512:1024],
                                in_=ot[:, 512:1024])
                        else:
                            nc.sync.dma_start(out=t["outp"][q0:q0 + 128, :],
                                              in_=ot)
                    return [c0, c1]
                chunks.extend(mk())
            return chunks

        # ---- schedule ----
        # Minimal prelude: only what round 0 of qb0/hp0 needs (heads 0-1's
        # Q and K). Everything else is round filler, so the ScalarE exp
        # stream — the secondary pacer — starts ~20us earlier.
        emit_qproj(0, 0)
        emit_kproj(0, 0)

        # qb0 filler maps (round index -> closures), deadline-correct for
        # the 2-round PV lag: S^T(sk)@round 2sk needs kproj(sk//2,0) in an
        # earlier round; PV(sk,h0)@round 2sk+2 needs vproj up to kt=2sk+1
        # by that round's filler slot (filler precedes the PV).
        sched_q0h0 = {
            0: [lambda: emit_vproj(0, 0)],
            1: [lambda: emit_vproj(0, 1)],
            2: [lambda: emit_vproj(0, 2)],
            3: [lambda: emit_kproj(1, 0)],
            4: [lambda: emit_vproj(0, 3)],
            5: [lambda: emit_vproj(1, 0)],
            6: [lambda: emit_vproj(1, 1)],
            7: [lambda: emit_kproj(2, 0)],
            8: [lambda: emit_vproj(1, 2), lambda: emit_vproj(1, 3)],
            9: [lambda: emit_vproj(2, 0)],
            10: [lambda: emit_vproj(2, 1)],
            11: [lambda: emit_kproj(3, 0)],
            12: [lambda: emit_vproj(2, 2), lambda: emit_vproj(2, 3)],
            13: [lambda: emit_vproj(3, 0)],
            14: [lambda: emit_vproj(3, 1)],
            15: [lambda: emit_vproj(3, 2), lambda: emit_vproj(3, 3)],
        }
        sched_q0h1 = {
            0: [lambda: emit_kproj(1, 1)],
            1: [lambda: emit_qproj(1, 0)],
            3: [lambda: emit_kproj(2, 1)],
            5: [lambda: emit_qproj(1, 1)],
            7: [lambda: emit_kproj(3, 1)],
            9: [lambda: emit_qproj(2, 0)],
        }

        def spread(items, n_rounds=32, reserve=2):
            """Assign items evenly to round indices, keeping `reserve`
            items back for the hp-tail boundaries."""
            body = items[:len(items) - reserve] if reserve else items
            tail = items[len(items) - reserve:] if reserve else []
            m = {}
            if body:
                for i, it in enumerate(body):
                    m.setdefault(i * n_rounds // len(body), []).append(it)
            return m, tail

        dmy_n = [0]

        def emit_dummy(tag):
            # HAM keepalive: occupies the PE during endgame dependency
            # stalls so the clock gate stays at 8/8 for the real matmuls
            dmy_n[0] += 1
            psd = ps.tile([128, 512], F32, tag=tag, bufs=2,
                          name=f"dmy_{dmy_n[0]}")
            nc.tensor.matmul(psd[:, 0:512], warm_w, warm_x,
                             start=True, stop=True)

        def run_qb(qb, s0, s1, t0, t1):
            for hp in range(2):
                sched = s0 if hp == 0 else s1
                tailf = t0 if hp == 0 else t1
                pso = [ps.tile([128, 512], F32, tag="acc", bufs=2,
                               name=f"pso_{qb}_{hp}_{hh}") for hh in range(2)]
                pend = []
                ridx = 0
                for sk in range(8):
                    for hh in range(2):
                        # order: S^T(r), filler, PV(r-2) — a 2-round PV lag
                        # so the PV's exp finished >1 round ago and the PE
                        # never waits on a fresh semaphore
                        pt = emit_st(qb, hp, sk, hh)
                        for fn in sched.pop(ridx, []):
                            fn()
                        if len(pend) == 2:
                            emit_pv(*pend.pop(0))
                        pend.append((qb, hp, sk, hh, pt, pso))
                        ridx += 1
                for p in pend:
                    emit_pv(*p)
                for fn in tailf:
                    fn()
                if qb == QT - 1:
                    emit_dummy("s")
                    emit_dummy("s")
                emit_hp_tail(qb, hp, pso)
                if qb == QT - 1:
                    for _ in range(3):
                        emit_dummy("acc")

        run_qb(0, sched_q0h0, sched_q0h1,
               [lambda: emit_qproj(0, 1), lambda: emit_kproj(0, 1)],
               [lambda: emit_qproj(2, 1)])
        post_tail = []
        for qb in range(1, QT):
            items = emit_outproj(qb - 1)
            if qb == 2:
                items += [lambda: emit_qproj(3, 0), lambda: emit_qproj(3, 1)]
            if qb == QT - 1:
                # hold two chunks back to bridge the PE through the very
                # last hp-tail chain (bc/recip/MULT) before outproj(qb3)
                post_tail = items[-2:]
                items = items[:-2]
            half0, rest = items[:len(items) // 2], items[len(items) // 2:]
            s0, t0 = spread(half0, n_rounds=16, reserve=1)
            s1, t1 = spread(rest, n_rounds=16, reserve=2 if qb == QT - 1 else 1)
            run_qb(qb, s0, s1, t0, t1)
        for ch in post_tail:
            ch()
        emit_dummy("acc")
        emit_dummy("acc")
        for ch in emit_outproj(QT - 1, deep=True):
            ch()


def _swizzle_a(aT):
    """[D, DG] -> [128, DC*DG]: partition p holds chunks c at (c, :)."""
    return np.ascontiguousarray(
        aT.reshape(DC, 128, DG).transpose(1, 0, 2).reshape(128, DC * DG))


def _pack_bo(boT):
    """[DG, D] -> [128, 2*D]: head-pair hp at cols hp*D, rows=pair dims."""
    out = boT.reshape(2, 128, D).transpose(1, 0, 2)
    return np.ascontiguousarray(out.reshape(128, 2 * D))


def _block_x(xT):
    """[D, L] -> [QT*128, DC*512] qt-major blocks, 8KB partition lines."""
    return np.ascontiguousarray(
        xT.reshape(DC, 128, QT, 512).transpose(2, 1, 0, 3).reshape(
            QT * 128, DC * 512))


_EYE = np.eye(64, dtype=np.float16)


def _prep_inputs(values, key, query, mask, Wv, Wk, Wq, Wo, bv, bk, bq):
    """Build the 8 per-core input maps (host-side shard + layout)."""
    xB = {}
    for n in range(N_BATCH):
        xB[("q", n)] = _block_x(query[n].T.astype(np.float16))
        xB[("k", n)] = _block_x(key[n].T.astype(np.float16))
        xB[("v", n)] = _block_x(values[n].T.astype(np.float16))
    in_maps = []
    for c in range(CORES):
        n, g = divmod(c, CORES // N_BATCH)
        rows = slice(g * DG, (g + 1) * DG)
        mrow = np.ascontiguousarray(
            mask[n, 0, 0, :].astype(np.float32).reshape(KT, 128).T)
        in_maps.append({
            "xqb": xB[("q", n)],
            "xkb": xB[("k", n)],
            "xvb": xB[("v", n)],
            "aq": _swizzle_a(Wq[rows, :].T.astype(np.float16)),
            "ak": _swizzle_a(Wk[rows, :].T.astype(np.float16)),
            "av": _swizzle_a(Wv[rows, :].T.astype(np.float16)),
            "bo": _pack_bo(Wo[:, rows].T.astype(np.float16)),
            "bq": np.ascontiguousarray(bq[None, rows].astype(np.float16)),
            "bk": np.ascontiguousarray(bk[None, rows].astype(np.float16)),
            "bv": np.ascontiguousarray(bv[None, rows].astype(np.float16)),
            "eye": _EYE,
            "maskf": mrow,
        })
    return in_maps


LAST_EXEC_NS = None
LAST_RES = None


def kernel(values, key, query, mask, Wv, bv, Wk, bk, Wq, bq, Wo, bo,
           trace=False, trace_cores=None):
    global LAST_EXEC_NS, LAST_RES
    values = np.asarray(values, dtype=np.float32)
    key = np.asarray(key, dtype=np.float32)
    query = np.asarray(query, dtype=np.float32)
    mask = np.asarray(mask)
    Wq, Wk, Wv, Wo = (np.asarray(Wq, np.float32), np.asarray(Wk, np.float32),
                      np.asarray(Wv, np.float32), np.asarray(Wo, np.float32))
    bq, bk, bv, bo = (np.asarray(bq, np.float32), np.asarray(bk, np.float32),
                      np.asarray(bv, np.float32), np.asarray(bo, np.float32))

    use_bias = bool(np.any(bq) or np.any(bk) or np.any(bv))
    use_mask = not bool(np.all(np.asarray(mask) == 1))

    nc = _build(use_bias, use_mask)
    in_maps = _prep_inputs(values, key, query, mask, Wv, Wk, Wq, Wo,
                           bv, bk, bq)
    kw = {}
    if trace_cores is not None:
        kw["trace_cores"] = trace_cores
    res = run_bass_kernel_spmd(nc, in_maps, core_ids=list(range(CORES)),
                               trace=trace, **kw)
    LAST_EXEC_NS = res.exec_time_ns
    LAST_RES = res

    out = np.zeros((N_BATCH, L, D), dtype=np.float32)
    for c in range(CORES):
        n = c // (CORES // N_BATCH)
        out[n] += res.results[c]["outp"].astype(np.float32)
    out += bo[None, None, :]
    return out


# revision 33
# speedup vs baseline: 1.0119x; 1.0119x over previous
"""Multi-head attention (N=2, L=2048, D=1024, H=16) on 8 NeuronCores.

Sharding: core c -> (batch n = c // 4, head group g = c % 4, 4 heads each).
Each core computes Q/K/V projections for its 4 heads, attention, and its
slice of the output projection. Host sums the 4 partial output projections
per batch and adds bo. HW exec ~210-213us (baseline was ~260-270us).

Design (v7), from iterative HW profiling:
- Host-blocked qt-major x layouts ([QT*128, DC*512], 8KB DMA lines) so
  inputs arrive in consumption order (the prior full-row layout starved
  PV of V until ~58us). The prior session's NaN-on-profiled-core fear
  about this layout did NOT reproduce.
- Minimal prelude (qproj(0,0)+kproj(0,0) only) then single-head rounds:
  per (qb, hp, sk, hh): S^T = 2 MMs into one [128,1024] fp32 psum tile
  (ring 2), one exp [128,1024] (ScalarE, scale=1/32 folded), PV = 2 MMs
  (M=65: V dims + ones column accumulating the softmax denominator).
  Ring granularity == round granularity keeps the exp stream fed.
- PV lags TWO rounds behind S^T so its exp finished >1 round earlier —
  the PE never waits on a fresh semaphore (this removed ~160ns/MM of
  exposed drain: PV means fell 253->222ns).
- Head-pair outer loop (hp): only 2 PV accumulators live -> PSUM fits:
  s(2x2 banks) + acc(2x1) + o(2x1) = 8 banks, leaving the "o" ring for
  interleaved filler matmuls. All projection/out-proj work is emitted as
  deadline-scheduled round fillers (PE FIFO = emission order, so every
  emitted op must have its deps already emitted).
- Out-proj packs head pairs: oN2[hp] [128,512] holds both heads' dims ->
  K=128 fully used, 8 MMs/qb instead of 16. The odd head's normalized
  rows are written by DVE tensor_tensor at partition offset 64 (works on
  this stack; single-partition DVE ops at an offset do NOT).
- Reciprocal on DVE: bc matmul broadcasts the RAW denominator row (K=1
  matmul from partition 64), reciprocal_approx_fast on [64,512] (the
  exact InstReciprocal is 5x slower: 3.2us/tile!), one tensor_tensor
  multiply. ScalarE runs exps only; no Ln/Exp act-table tricks needed.
- HAM keepalive: warmup matmuls on memset tiles during the DMA lead-in,
  plus dummy matmuls bridging the endgame dependency stalls (without
  them the final out-proj runs at the cold 427ns clock, +3.4us).
- Endgame out-proj alternates psum tags "o"/"s" (4 tiles in flight) and
  CASTs alternate DVE/ScalarE so the copy-out is not DVE-bound.
Remaining structure: PE busy ~186us (floor ~174-177: S^T and PV are
output/stream-bound at N=512; fp8 rejected — P-noise transfers ~1:1 to
output rel err, ~1.3-3% vs the 2e-2 gate), ScalarE exp stream 135us,
~8us runtime head + ~7us teardown tail are fixed. Exp batching to
[128,2048] would save ~14us ScalarE but needs 2 more PSUM banks than
exist. Run-to-run variance is +-1.5% (HAM phase / power states).
"""
import os
import sys
import types

import numpy as np

N_BATCH = 2
L = 2048
D = 1024
H = 16
HD = 64
CORES = 8
GH = 4            # heads per core
DG = GH * HD      # 256 = projected dims per core
QB = 512          # q block
KT = L // 128     # 16 k tiles
QT = L // QB      # 4 q blocks
DC = D // 128     # 8 din chunks
SCALE = 1.0 / 32.0  # 1/sqrt(D)
SHIFT_MODE = "dve"  # "dve": direct offset write; "pe": identity matmul shift


def _install_ntff_hook():
    """The image's antenv stub lacks axon_hooks; shim it so trace=True works."""
    if "antenv.axon_hooks" in sys.modules:
        return
    mod = types.ModuleType("antenv.axon_hooks")
    mod._hook = None
    mod.set_axon_ntff_profile_hook = lambda h: setattr(mod, "_hook", h)
    mod.get_axon_ntff_profile_hook = lambda: mod._hook
    sys.modules["antenv.axon_hooks"] = mod
    try:
        from trn_agent_boot.trn_boot import _ntff_profile_via_ctypes
        mod._hook = _ntff_profile_via_ctypes("/opt/axon/libaxon_pjrt.so")
    except Exception:
        mod._hook = None


_install_ntff_hook()

import concourse.bacc as bacc
import concourse.mybir as mybir
import concourse.tile as tile
from concourse.bass_utils import run_bass_kernel_spmd

F32 = mybir.dt.float32
F16 = mybir.dt.float16
AF = mybir.ActivationFunctionType
MULT = mybir.AluOpType.mult

_CACHE = {}


def _build(use_bias, use_mask):
    key = (use_bias, use_mask)
    if key in _CACHE:
        return _CACHE[key]

    nc = bacc.Bacc("TRN2", debug=False, num_devices=CORES)

    xqb = nc.dram_tensor("xqb", [QT * 128, DC * 512], F16, kind="ExternalInput").ap()
    xkb = nc.dram_tensor("xkb", [QT * 128, DC * 512], F16, kind="ExternalInput").ap()
    xvb = nc.dram_tensor("xvb", [QT * 128, DC * 512], F16, kind="ExternalInput").ap()
    aq = nc.dram_tensor("aq", [128, DC * DG], F16, kind="ExternalInput").ap()
    ak = nc.dram_tensor("ak", [128, DC * DG], F16, kind="ExternalInput").ap()
    av = nc.dram_tensor("av", [128, DC * DG], F16, kind="ExternalInput").ap()
    bo = nc.dram_tensor("bo", [128, 2 * D], F16, kind="ExternalInput").ap()
    bq = nc.dram_tensor("bq", [1, DG], F16, kind="ExternalInput").ap()
    bk = nc.dram_tensor("bk", [1, DG], F16, kind="ExternalInput").ap()
    bv = nc.dram_tensor("bv", [1, DG], F16, kind="ExternalInput").ap()
    eye = nc.dram_tensor("eye", [64, 64], F16, kind="ExternalInput").ap()
    maskf = nc.dram_tensor("maskf", [128, KT], F32, kind="ExternalInput").ap()
    outp = nc.dram_tensor("outp", [L, D], F16, kind="ExternalOutput").ap()

    with tile.TileContext(nc) as tc:
        _emit(nc, tc, dict(xqb=xqb, xkb=xkb, xvb=xvb, aq=aq, ak=ak, av=av,
                           bo=bo, bq=bq, bk=bk, bv=bv, eye=eye, maskf=maskf,
                           outp=outp),
              use_bias, use_mask)
    nc.compile()
    _CACHE[key] = nc
    return nc


def _emit(nc, tc, t, use_bias, use_mask):
    from contextlib import ExitStack
    ctx = ExitStack()
    with ctx:
        sb_w = ctx.enter_context(tc.tile_pool(name="sb_w", bufs=1))
        sb_qkv = ctx.enter_context(tc.tile_pool(name="sb_qkv", bufs=1))
        sb_pt = ctx.enter_context(tc.tile_pool(name="sb_pt", bufs=4))
        sb_n = ctx.enter_context(tc.tile_pool(name="sb_n", bufs=4))
        sb_out = ctx.enter_context(tc.tile_pool(name="sb_out", bufs=3))
        ps = ctx.enter_context(tc.tile_pool(name="ps", bufs=2, space="PSUM"))

        # ---- resident tiles ----
        aq_t = sb_w.tile([128, DC, DG], F16, tag="aq")
        ak_t = sb_w.tile([128, DC, DG], F16, tag="ak")
        av_t = sb_w.tile([128, DC, DG], F16, tag="av")
        bo_t = sb_w.tile([128, 2, D], F16, tag="bo")
        ones_t = sb_w.tile([128, 512], F16, tag="ones")
        eye_t = sb_w.tile([64, 64], F16, tag="eye")
        xq_res = sb_w.tile([128, QT, DC * 512], F16, tag="xq")
        xk_res = sb_w.tile([128, QT, DC * 512], F16, tag="xk")
        xv_res = sb_w.tile([128, QT, DC * 512], F16, tag="xv")
        KT_sb = [sb_qkv.tile([128, L], F16, tag=f"kt{m}", name=f"KTm{m}")
                 for m in range(2)]
        QT_z = [sb_qkv.tile([128, L], F16, tag=f"qz{h}", name=f"QTz{h}")
                for h in range(GH)]
        V1 = sb_qkv.tile([128, KT, GH, HD + 1], F16, tag="v1")
        # oN2[qb%2][hp]: packed normalized heads for the out-projection
        oN2 = [[sb_qkv.tile([128, 512], F16, tag=f"oN{b}{hp}",
                            name=f"oN{b}{hp}") for hp in range(2)]
               for b in range(2)]

        # ---- warmup tiles (no DMA deps; HAM ramp during input stream) ----
        warm_w = sb_w.tile([128, 128], F16, tag="warmw")
        warm_x = sb_w.tile([128, 512], F16, tag="warmx")
        nc.vector.memset(warm_w, 0.0)
        nc.vector.memset(warm_x, 0.0)
        nc.vector.memset(ones_t, 1.0)
        # ACT table warmup (exp)
        warm_a = sb_w.tile([1, 32], F32, tag="warma")
        nc.vector.memset(warm_a, 1.0)
        warm_b = sb_w.tile([1, 32], F32, tag="warmb")
        nc.scalar.activation(out=warm_b, in_=warm_a, func=AF.Exp)

        for h in range(GH):
            z0 = 0 if h % 2 else 64
            nc.vector.memset(QT_z[h][z0:z0 + 64, :], 0.0)

        # ---- input DMAs: one priority-ordered queue (sync) ----
        # First tiles split in halves so qproj/kproj start ~2us earlier.
        def dma_x(res, src, qt, half=None):
            if half is None:
                nc.sync.dma_start(out=res[:, qt, :],
                                  in_=src[qt * 128:(qt + 1) * 128, :])
            else:
                h0 = half * (DC // 2) * 512
                h1 = h0 + (DC // 2) * 512
                nc.sync.dma_start(out=res[:, qt, h0:h1],
                                  in_=src[qt * 128:(qt + 1) * 128, h0:h1])

        dma_x(xq_res, t["xqb"], 0, 0)
        nc.sync.dma_start(out=aq_t, in_=t["aq"].rearrange("p (c d) -> p c d", c=DC))
        dma_x(xq_res, t["xqb"], 0, 1)
        nc.sync.dma_start(out=ak_t, in_=t["ak"].rearrange("p (c d) -> p c d", c=DC))
        dma_x(xk_res, t["xkb"], 0, 0)
        dma_x(xk_res, t["xkb"], 0, 1)
        nc.sync.dma_start(out=av_t, in_=t["av"].rearrange("p (c d) -> p c d", c=DC))
        dma_x(xv_res, t["xvb"], 0, 0)
        dma_x(xv_res, t["xvb"], 0, 1)
        for qt in range(1, QT):
            dma_x(xk_res, t["xkb"], qt)
            dma_x(xv_res, t["xvb"], qt)
            dma_x(xq_res, t["xqb"], qt)
        nc.sync.dma_start(out=bo_t, in_=t["bo"].rearrange("p (a d) -> p a d", a=2))
        if SHIFT_MODE == "pe":
            nc.sync.dma_start(out=eye_t, in_=t["eye"])
        if use_mask:
            mask_t = sb_w.tile([128, KT], F32, tag="mask")
            nc.sync.dma_start(out=mask_t, in_=t["maskf"])
        bq_t = bk_t = bv_t = None
        if use_bias:
            bq_t = sb_w.tile([1, DG], F16, tag="bq")
            bk_t = sb_w.tile([1, DG], F16, tag="bk")
            bv_t = sb_w.tile([1, DG], F16, tag="bv")
            nc.sync.dma_start(out=bq_t, in_=t["bq"])
            nc.sync.dma_start(out=bk_t, in_=t["bk"])
            nc.sync.dma_start(out=bv_t, in_=t["bv"])

        # V1 ones column (column HD of every (kt, h) slot)
        if use_mask:
            ones4 = sb_w.tile([128, GH], F32, tag="ones4")
            nc.vector.memset(ones4, 1.0)
            for kt in range(KT):
                nc.vector.tensor_scalar_mul(
                    V1[:, kt, :, HD:HD + 1],
                    ones4.rearrange("p h -> p h 1"), mask_t[:, kt:kt + 1])
        else:
            nc.vector.memset(V1[:, :, :, HD:HD + 1], 1.0)

        # ---- PE warmup: dummy matmuls to ramp HAM while inputs stream ----
        for w in range(10):
            psw = ps.tile([128, 512], F32, tag="o", bufs=2, name=f"psw_{w}")
            nc.tensor.matmul(psw[:, 0:512], warm_w, warm_x,
                             start=True, stop=True)

        # ---- emit helpers ----
        def emit_qproj(qb, p):
            # packed head pair p: one M=128 matmul per c chunk
            psq = ps.tile([128, 512], F32, tag="o", bufs=2, name=f"psq_{qb}_{p}")
            for c in range(DC):
                xsl = xq_res[:, qb, c * 512:(c + 1) * 512]
                nc.tensor.matmul(
                    psq[:, 0:512], aq_t[:, c, p * 128:(p + 1) * 128], xsl,
                    start=(c == 0), stop=(c == DC - 1 and not use_bias))
            if use_bias:
                nc.tensor.matmul(
                    psq[:, 0:512], bq_t[:, p * 128:(p + 1) * 128],
                    ones_t[0:1, :], start=False, stop=True)
            for hh in range(2):
                h = p * 2 + hh
                r0 = 64 * hh
                nc.vector.tensor_copy(
                    QT_z[h][r0:r0 + 64, qb * 512:(qb + 1) * 512],
                    psq[r0:r0 + 64, 0:512])

        def emit_kproj(qt, m):
            psm = ps.tile([128, 512], F32, tag="o", bufs=2, name=f"psk_{qt}_{m}")
            for c in range(DC):
                xsl = xk_res[:, qt, c * 512:(c + 1) * 512]
                nc.tensor.matmul(
                    psm[:, 0:512], ak_t[:, c, m * 128:(m + 1) * 128], xsl,
                    start=(c == 0), stop=(c == DC - 1 and not use_bias))
            if use_bias:
                nc.tensor.matmul(
                    psm[:, 0:512], bk_t[:, m * 128:(m + 1) * 128],
                    ones_t[0:1, :], start=False, stop=True)
            nc.vector.tensor_copy(
                KT_sb[m][:, qt * 512:(qt + 1) * 512], psm[:, 0:512])

        def emit_vproj(ktg, j):
            psv = ps.tile([128, 512], F32, tag="o", bufs=2, name=f"psv_{ktg}_{j}")
            for c in range(DC):
                xsl = xv_res[:, ktg, c * 512:(c + 1) * 512]
                nc.tensor.matmul(
                    psv[:, 0:DG], xsl[:, j * 128:(j + 1) * 128],
                    av_t[:, c, :],
                    start=(c == 0), stop=(c == DC - 1 and not use_bias))
            if use_bias:
                nc.tensor.matmul(
                    psv[:, 0:DG], ones_t[0:1, 0:128], bv_t,
                    start=False, stop=True)
            kt = ktg * 4 + j
            srcv = psv[:, 0:DG].rearrange("p (h d) -> p h d", h=GH)
            if use_mask:
                nc.vector.tensor_scalar_mul(
                    V1[:, kt, :, 0:HD], srcv, mask_t[:, kt:kt + 1])
            else:
                nc.vector.tensor_copy(V1[:, kt, :, 0:HD], srcv)

        # ---- the attention round engine ----
        # round r = (qb, hp, sk, hh): S^T (2 MMs -> pss), exp, then PV of
        # the PREVIOUS round (software pipeline, 1-round lag).
        filler = []           # list of closures, each ~2 matmuls
        fill_debt = [0.0]     # fractional chunks owed

        def pop_filler(n=1.0):
            fill_debt[0] += n
            while fill_debt[0] >= 1.0 and filler:
                filler.pop(0)()
                fill_debt[0] -= 1.0

        def emit_st(qb, hp, sk, hh):
            h = hp * 2 + hh
            pss = ps.tile([128, 1024], F32, tag="s", bufs=2,
                          name=f"pss_{qb}_{sk}_{h}")
            for dk in range(2):
                kt = sk * 2 + dk
                nc.tensor.matmul(
                    pss[:, dk * 512:(dk + 1) * 512],
                    KT_sb[hp][:, kt * 128:(kt + 1) * 128],
                    QT_z[h][:, qb * 512:qb * 512 + QB],
                    start=True, stop=True)
            pt = sb_pt.tile([128, 1024], F16, tag="pt", bufs=6,
                            name=f"pt_{qb}_{sk}_{h}")
            nc.scalar.activation(out=pt, in_=pss, func=AF.Exp, scale=SCALE)
            return pt

        def emit_pv(qb, hp, sk, hh, pt, pso):
            h = hp * 2 + hh
            for dk in range(2):
                kt = sk * 2 + dk
                nc.tensor.matmul(
                    pso[hh][0:HD + 1, :], V1[:, kt, h, :],
                    pt[:, dk * 512:(dk + 1) * 512],
                    start=(kt == 0), stop=(kt == KT - 1))

        def emit_hp_tail(qb, hp, pso):
            # normalize both heads of the pair into oN2[qb%2][hp];
            # the two heads' chains are interleaved so engines pipeline
            on = oN2[qb % 2][hp]
            oTs, bcs, rcps = [], [], []
            for hh in range(2):
                oT = sb_n.tile([HD + 1, 512], F16, tag="oT", bufs=4,
                               name=f"oT_{qb}_{hp}_{hh}")
                nc.vector.tensor_copy(oT, pso[hh][0:HD + 1, :])
                oTs.append(oT)
            for hh in range(2):
                bc = ps.tile([128, 512], F32, tag="o", bufs=2,
                             name=f"bc_{qb}_{hp}_{hh}")
                nc.tensor.matmul(
                    bc[0:64, :], ones_t[64:65, 0:64], oTs[hh][64:65, :],
                    start=True, stop=True, tile_position=(64, 0))
                bcs.append(bc)
            for hh in range(2):
                rcp = sb_n.tile([64, 512], F32, tag="rcp", bufs=2,
                                name=f"rcp_{qb}_{hp}_{hh}")
                nc.vector.reciprocal_approx_fast(out=rcp, in_=bcs[hh][0:64, :])
                rcps.append(rcp)
            for hh in range(2):
                rows = slice(0, 64) if hh == 0 else slice(64, 128)
                nc.vector.tensor_tensor(on[rows, :], oTs[hh][0:64, :],
                                        rcps[hh], op=MULT)

        def emit_outproj_chunk(qb, mq, nb, psout_box, tag="o"):
            # two packed MMs (hp 0,1) accumulating psout, then CAST out
            on_pair = oN2[qb % 2]
            psout = ps.tile([128, 512], F32, tag=tag, bufs=2,
                            name=f"psout_{qb}_{mq}_{nb}")
            for hp in range(2):
                nc.tensor.matmul(
                    psout[:, 0:512],
                    on_pair[hp][:, mq * 128:(mq + 1) * 128],
                    bo_t[:, hp, nb * 512:(nb + 1) * 512],
                    start=(hp == 0), stop=(hp == 1))
            psout_box[nb] = psout

        def emit_outproj(qb, deep=False):
            # returns filler closures: 8 chunks of 2 MMs + CAST/DMA.
            # deep=True (endgame): alternate psum tags "o"/"s" so 4 psout
            # tiles pipeline (the attention rings are drained by then).
            chunks = []
            for mq in range(4):
                ot = sb_out.tile([128, D], F16, tag="ot", name=f"ot_{qb}_{mq}")
                box = {}
                tg0 = "s" if deep and mq % 2 else "o"
                tg1 = "s" if deep and not mq % 2 else "o"

                def mk(qb=qb, mq=mq, ot=ot, box=box, tg0=tg0, tg1=tg1):
                    # endgame CASTs alternate ScalarE/DVE (ScalarE's exp
                    # stream is drained by then) so the copy-out pipelines
                    def cp(dst, src):
                        if deep and mq % 2:
                            nc.scalar.copy(out=dst, in_=src)
                        else:
                            nc.vector.tensor_copy(dst, src)

                    def c0():
                        emit_outproj_chunk(qb, mq, 0, box, tg0)
                        cp(ot[:, 0:512], box[0][:, 0:512])
                        if deep:
                            # endgame: ship each half as soon as it's cast
                            q0 = qb * QB + mq * 128
                            nc.sync.dma_start(
                                out=t["outp"][q0:q0 + 128, 0:512],
                                in_=ot[:, 0:512])

                    def c1():
                        emit_outproj_chunk(qb, mq, 1, box, tg1)
                        cp(ot[:, 512:1024], box[1][:, 0:512])
                        q0 = qb * QB + mq * 128
                        if deep:
                            nc.sync.dma_start(
                                out=t["outp"][q0:q0 + 128, 512:1024],
                                in_=ot[:, 512:1024])
                        else:
                            nc.sync.dma_start(out=t["outp"][q0:q0 + 128, :],
                                              in_=ot)
                    return [c0, c1]
                chunks.extend(mk())
            return chunks

        # ---- schedule ----
        # Minimal prelude: only what round 0 of qb0/hp0 needs (heads 0-1's
        # Q and K). Everything else is round filler, so the ScalarE exp
        # stream — the secondary pacer — starts ~20us earlier.
        emit_qproj(0, 0)
        emit_kproj(0, 0)

        # qb0 filler maps (round index -> closures), deadline-correct for
        # the 2-round PV lag: S^T(sk)@round 2sk needs kproj(sk//2,0) in an
        # earlier round; PV(sk,h0)@round 2sk+2 needs vproj up to kt=2sk+1
        # by that round's filler slot (filler precedes the PV).
        sched_q0h0 = {
            0: [lambda: emit_vproj(0, 0)],
            1: [lambda: emit_vproj(0, 1)],
            2: [lambda: emit_vproj(0, 2)],
            3: [lambda: emit_kproj(1, 0)],
            4: [lambda: emit_vproj(0, 3)],
            5: [lambda: emit_vproj(1, 0)],
            6: [lambda: emit_vproj(1, 1)],
            7: [lambda: emit_kproj(2, 0)],
            8: [lambda: emit_vproj(1, 2), lambda: emit_vproj(1, 3)],
            9: [lambda: emit_vproj(2, 0)],
            10: [lambda: emit_vproj(2, 1)],
            11: [lambda: emit_kproj(3, 0)],
            12: [lambda: emit_vproj(2, 2), lambda: emit_vproj(2, 3)],
            13: [lambda: emit_vproj(3, 0)],
            14: [lambda: emit_vproj(3, 1)],
            15: [lambda: emit_vproj(3, 2), lambda: emit_vproj(3, 3)],
        }
        sched_q0h1 = {
            0: [lambda: emit_kproj(1, 1)],
            1: [lambda: emit_qproj(1, 0)],
            3: [lambda: emit_kproj(2, 1)],
            5: [lambda: emit_qproj(1, 1)],
            7: [lambda: emit_kproj(3, 1)],
            9: [lambda: emit_qproj(2, 0)],
        }

        def spread(items, n_rounds=32, reserve=2):
            """Assign items evenly to round indices, keeping `reserve`
            items back for the hp-tail boundaries."""
            body = items[:len(items) - reserve] if reserve else items
            tail = items[len(items) - reserve:] if reserve else []
            m = {}
            if body:
                for i, it in enumerate(body):
                    m.setdefault(i * n_rounds // len(body), []).append(it)
            return m, tail

        dmy_n = [0]

        def emit_dummy(tag):
            # HAM keepalive: occupies the PE during endgame dependency
            # stalls so the clock gate stays at 8/8 for the real matmuls
            dmy_n[0] += 1
            psd = ps.tile([128, 512], F32, tag=tag, bufs=2,
                          name=f"dmy_{dmy_n[0]}")
            nc.tensor.matmul(psd[:, 0:512], warm_w, warm_x,
                             start=True, stop=True)

        def run_qb(qb, s0, s1, t0, t1):
            for hp in range(2):
                sched = s0 if hp == 0 else s1
                tailf = t0 if hp == 0 else t1
                pso = [ps.tile([128, 512], F32, tag="acc", bufs=2,
                               name=f"pso_{qb}_{hp}_{hh}") for hh in range(2)]
                pend = []
                ridx = 0
                for sk in range(8):
                    for hh in range(2):
                        # order: S^T(r), filler, PV(r-2) — a 2-round PV lag
                        # so the PV's exp finished >1 round ago and the PE
                        # never waits on a fresh semaphore
                        pt = emit_st(qb, hp, sk, hh)
                        for fn in sched.pop(ridx, []):
                            fn()
                        if len(pend) == 2:
                            emit_pv(*pend.pop(0))
                        pend.append((qb, hp, sk, hh, pt, pso))
                        ridx += 1
                for p in pend:
                    emit_pv(*p)
                for fn in tailf:
                    fn()
                if qb == QT - 1:
                    emit_dummy("s")
                    emit_dummy("s")
                emit_hp_tail(qb, hp, pso)
                if qb == QT - 1:
                    for _ in range(3):
                        emit_dummy("acc")

        run_qb(0, sched_q0h0, sched_q0h1,
               [lambda: emit_qproj(0, 1), lambda: emit_kproj(0, 1)],
               [lambda: emit_qproj(2, 1)])
        post_tail = []
        for qb in range(1, QT):
            items = emit_outproj(qb - 1)
            if qb == 2:
                items += [lambda: emit_qproj(3, 0), lambda: emit_qproj(3, 1)]
            if qb == QT - 1:
                # hold two chunks back to bridge the PE through the very
                # last hp-tail chain (bc/recip/MULT) before outproj(qb3)
                post_tail = items[-2:]
                items = items[:-2]
            half0, rest = items[:len(items) // 2], items[len(items) // 2:]
            s0, t0 = spread(half0, n_rounds=16, reserve=1)
            s1, t1 = spread(rest, n_rounds=16, reserve=2 if qb == QT - 1 else 1)
            run_qb(qb, s0, s1, t0, t1)
        for ch in post_tail:
            ch()
        emit_dummy("acc")
        emit_dummy("acc")
        for ch in emit_outproj(QT - 1, deep=True):
            ch()


def _swizzle_a(aT):
    """[D, DG] -> [128, DC*DG]: partition p holds chunks c at (c, :)."""
    return np.ascontiguousarray(
        aT.reshape(DC, 128, DG).transpose(1, 0, 2).reshape(128, DC * DG))


def _pack_bo(boT):
    """[DG, D] -> [128, 2*D]: head-pair hp at cols hp*D, rows=pair dims."""
    out = boT.reshape(2, 128, D).transpose(1, 0, 2)
    return np.ascontiguousarray(out.reshape(128, 2 * D))


def _block_x(xT):
    """[D, L] -> [QT*128, DC*512] qt-major blocks, 8KB partition lines."""
    return np.ascontiguousarray(
        xT.reshape(DC, 128, QT, 512).transpose(2, 1, 0, 3).reshape(
            QT * 128, DC * 512))


_EYE = np.eye(64, dtype=np.float16)


def _prep_inputs(values, key, query, mask, Wv, Wk, Wq, Wo, bv, bk, bq):
    """Build the 8 per-core input maps (host-side shard + layout)."""
    xB = {}
    for n in range(N_BATCH):
        xB[("q", n)] = _block_x(query[n].T.astype(np.float16))
        xB[("k", n)] = _block_x(key[n].T.astype(np.float16))
        xB[("v", n)] = _block_x(values[n].T.astype(np.float16))
    in_maps = []
    for c in range(CORES):
        n, g = divmod(c, CORES // N_BATCH)
        rows = slice(g * DG, (g + 1) * DG)
        mrow = np.ascontiguousarray(
            mask[n, 0, 0, :].astype(np.float32).reshape(KT, 128).T)
        in_maps.append({
            "xqb": xB[("q", n)],
            "xkb": xB[("k", n)],
            "xvb": xB[("v", n)],
            "aq": _swizzle_a(Wq[rows, :].T.astype(np.float16)),
            "ak": _swizzle_a(Wk[rows, :].T.astype(np.float16)),
            "av": _swizzle_a(Wv[rows, :].T.astype(np.float16)),
            "bo": _pack_bo(Wo[:, rows].T.astype(np.float16)),
            "bq": np.ascontiguousarray(bq[None, rows].astype(np.float16)),
            "bk": np.ascontiguousarray(bk[None, rows].astype(np.float16)),
            "bv": np.ascontiguousarray(bv[None, rows].astype(np.float16)),
            "eye": _EYE,
            "maskf": mrow,
        })
    return in_maps


LAST_EXEC_NS = None
LAST_RES = None


def kernel(values, key, query, mask, Wv, bv, Wk, bk, Wq, bq, Wo, bo,
           trace=False, trace_cores=None):
    global LAST_EXEC_NS, LAST_RES
    values = np.asarray(values, dtype=np.float32)
    key = np.asarray(key, dtype=np.float32)
    query = np.asarray(query, dtype=np.float32)
    mask = np.asarray(mask)
    Wq, Wk, Wv, Wo = (np.asarray(Wq, np.float32), np.asarray(Wk, np.float32),
                      np.asarray(Wv, np.float32), np.asarray(Wo, np.float32))
    bq, bk, bv, bo = (np.asarray(bq, np.float32), np.asarray(bk, np.float32),
                      np.asarray(bv, np.float32), np.asarray(bo, np.float32))

    use_bias = bool(np.any(bq) or np.any(bk) or np.any(bv))
    use_mask = not bool(np.all(np.asarray(mask) == 1))

    nc = _build(use_bias, use_mask)
    in_maps = _prep_inputs(values, key, query, mask, Wv, Wk, Wq, Wo,
                           bv, bk, bq)
    kw = {}
    if trace_cores is not None:
        kw["trace_cores"] = trace_cores
    res = run_bass_kernel_spmd(nc, in_maps, core_ids=list(range(CORES)),
                               trace=trace, **kw)
    LAST_EXEC_NS = res.exec_time_ns
    LAST_RES = res

    out = np.zeros((N_BATCH, L, D), dtype=np.float32)
    for c in range(CORES):
        n = c // (CORES // N_BATCH)
        out[n] += res.results[c]["outp"].astype(np.float32)
    out += bo[None, None, :]
    return out
